# revision 25
# baseline (speedup 1.0000x reference)
"""Multi-head cross-attention Trainium2 kernel (8 NeuronCores, SPMD).

Problem: nn_MultiHeadCrossAttention_31791347925263
  x:[4,2048,768], y:[4,2048,768], 12 heads x 64, fp32.
  out = softmax((x Wq^T)(y Wk^T)^T / 8 + mask) (y Wv^T) Wo^T   (+ zero biases)

Sharding: 8 cores = (batch b in 0..3) x (query half in 0..1). Each core
computes the full attention for its 1024 query rows against all 2048 keys
of its batch. No collectives; outputs concatenate.

Design (v3, ACT-limited pipeline, all-bf16):
  Measured HW laws driving this shape:
  - A K=128 matmul streams 512 moving cols in ~226ns (full 2.4GHz);
    K<=64 matmuls run exactly 2x slower. So QK (contraction = head_dim
    = 64) uses ZERO-PADDED stationaries: kTz[h] is [128, 2048] with the
    head's k in rows 0-63 and zeros in 64-127; the moving qT block has
    the sibling head's (finite) q in rows 64-127, killed by the zeros.
    This halves QK's PE time.
  - fp8 anywhere in the PV chain costs ~2-3% output error (softmax
    output rel err ~= per-element rel err of P~/v; it does NOT average
    down), so everything stays bf16 (~0.7% total).
  - The Scalar engine exp (25.2M scores -> 192 x [128,1024] ACTIVATEs
    at ~1.1us) is a ~214us floor. PE work (614k cols ~= 256us at full
    clock) is brought to ~82+82+15us attention-side by the K=128 trick,
    with the 77us of kT/qT/v' projections injected into per-pair PE
    slack so ACT never starves. PSUM: 2x QK score slots [128,1024] (4
    banks) + PV accumulator [65,1024] (2) + 2 projection slots (2).
  - Single-head pipeline with QK emitted 2 key-blocks ahead of PV so
    the in-order PE queue never blocks on exp; projection chunks are
    force-drained before their consumers (deadlock safety).
  - PV's 65th stationary column (ones) accumulates the softmax
    denominator free; normalize = DVE copy + reciprocal + gpsimd
    partition-broadcast + DVE mul into separate bf16 vnorm tiles.
  - Output projection (bf16) at the end over 3 rotating PSUM slots.
"""

import numpy as np

B, S, D = 4, 2048, 768
H, Dh = 12, 64
SQ = S // 2          # queries per core
N_CORES = 8
DB = D // 128        # 6 d_model blocks
SKB = S // 128       # 16 key blocks
SQB = SQ // 128      # 8 query blocks per core
VPW = H * (Dh + 1)   # 780: v' width (64 v cols + 1 ones col per head)

_cache = {}


def _build_nc():
    import concourse.mybir as mybir
    import concourse.tile as tile
    from concourse import bacc

    f32 = mybir.dt.float32
    bf16 = mybir.dt.bfloat16
    EXP = mybir.ActivationFunctionType.Exp

    nc = bacc.Bacc("TRN2", target_bir_lowering=False)
    x16 = nc.dram_tensor("x16", [128, DB, SQ], bf16, kind="ExternalInput")
    y16 = nc.dram_tensor("y16", [128, DB, S], bf16, kind="ExternalInput")
    wq16 = nc.dram_tensor("wq16", [128, DB, D], bf16, kind="ExternalInput")
    wk16 = nc.dram_tensor("wk16", [128, DB, D], bf16, kind="ExternalInput")
    wv16 = nc.dram_tensor("wv16", [128, DB, D], bf16, kind="ExternalInput")
    wo16 = nc.dram_tensor("wo16", [128, DB, D], bf16, kind="ExternalInput")
    out = nc.dram_tensor("out", [SQ, D], f32, kind="ExternalOutput")

    with tile.TileContext(nc) as tc:
        with tc.tile_pool(name="persist", bufs=1) as pp, \
             tc.tile_pool(name="mmps", bufs=2, space="PSUM") as mm_ps, \
             tc.tile_pool(name="vtps", bufs=1, space="PSUM") as vt_ps, \
             tc.tile_pool(name="pjps", bufs=2, space="PSUM") as pj_ps, \
             tc.tile_pool(name="pt16p", bufs=4) as pt_pool, \
             tc.tile_pool(name="nrm", bufs=1) as nrm_pool, \
             tc.tile_pool(name="osb", bufs=3) as o_pool:

            y16t = pp.tile([128, DB, S], bf16, name="y16t")
            wk16t = pp.tile([128, DB, D], bf16, name="wk16t")
            x16t = pp.tile([128, DB, SQ], bf16, name="x16t")
            wq16t = pp.tile([128, DB, D], bf16, name="wq16t")
            wv16t = pp.tile([128, DB, D], bf16, name="wv16t")
            wo16t = pp.tile([128, DB, D], bf16, name="wo16t")

            # zero-padded per-head k: rows 0-63 = head's kT, 64-127 = 0
            kTz = [pp.tile([128, S], bf16, name=f"kTz{i}") for i in range(H)]
            qT = [pp.tile([128, SQ], bf16, name=f"qT{i}") for i in range(DB)]
            vnorm = [pp.tile([128, SQ], bf16, name=f"vn{i}")
                     for i in range(DB)]
            vp16 = [pp.tile([128, VPW], bf16, name=f"vp16_{i}")
                    for i in range(SKB)]
            vp3 = [t.rearrange("p (h c) -> p h c", c=Dh + 1) for t in vp16]

            # ---- input DMA, priority order ----
            nc.sync.dma_start(out=wk16t, in_=wk16[:, :, :])
            for kb in range(DB):
                nc.sync.dma_start(out=y16t[:, kb, :], in_=y16[:, kb, :])
            nc.sync.dma_start(out=wq16t, in_=wq16[:, :, :])
            for kb in range(DB):
                nc.sync.dma_start(out=x16t[:, kb, :], in_=x16[:, kb, :])
            nc.sync.dma_start(out=wv16t, in_=wv16[:, :, :])
            nc.sync.dma_start(out=wo16t, in_=wo16[:, :, :])

            # head h's k occupies the same partition rows as its q in qT:
            # even heads rows 0-63 (zeros 64-127), odd heads rows 64-127
            for h in range(H):
                z0 = 64 if h % 2 == 0 else 0
                nc.gpsimd.memset(kTz[h][z0:z0 + 64, :], 0.0)
            for skb in range(SKB):
                nc.vector.memset(vp3[skb][:, :, Dh], 1.0)

            # ---- projection chunk emitters ----
            def emit_kt_chunk(ob, c4):
                ps = pj_ps.tile([128, 512], f32, name="pjps", tag="pjps")
                for kb in range(DB):
                    nc.tensor.matmul(
                        ps[:, :],
                        wk16t[:, kb, ob * 128:(ob + 1) * 128],
                        y16t[:, kb, c4 * 512:(c4 + 1) * 512],
                        start=(kb == 0), stop=(kb == DB - 1))
                cols = slice(c4 * 512, (c4 + 1) * 512)
                nc.vector.tensor_copy(kTz[2 * ob][0:64, cols], ps[0:64, :])
                nc.vector.tensor_copy(kTz[2 * ob + 1][64:128, cols],
                                      ps[64:128, :])

            def emit_qt_chunk(ob, c2):
                ps = pj_ps.tile([128, 512], f32, name="pjps", tag="pjps")
                for kb in range(DB):
                    nc.tensor.matmul(
                        ps[:, :],
                        wq16t[:, kb, ob * 128:(ob + 1) * 128],
                        x16t[:, kb, c2 * 512:(c2 + 1) * 512],
                        start=(kb == 0), stop=(kb == DB - 1))
                nc.vector.tensor_copy(
                    qT[ob][:, c2 * 512:(c2 + 1) * 512], ps[:, :])

            def emit_vp_chunk(skb, c):
                ps = pj_ps.tile([128, 512], f32, name="pjps", tag="pjps")
                for kb in range(DB):
                    nc.tensor.matmul(
                        ps[:, 0:384],
                        y16t[:, kb, skb * 128:(skb + 1) * 128],
                        wv16t[:, kb, c * 384:(c + 1) * 384],
                        start=(kb == 0), stop=(kb == DB - 1))
                src = ps[:, 0:384].rearrange("p (h c) -> p h c", c=Dh)
                nc.vector.tensor_copy(
                    vp3[skb][:, c * 6:(c + 1) * 6, 0:Dh], src)

            # task queue: (tag, mm_count, emit_fn), in need-by order.
            # vp chunk-0 tasks are NOT queued: head 0's PV forces them JIT.
            tasks = []

            def queue_ob(ob):
                for c4 in range(4):
                    tasks.append((("kt", ob), 6,
                                  lambda ob=ob, c4=c4: emit_kt_chunk(ob, c4)))
                for c2 in range(2):
                    tasks.append((("qt", ob), 6,
                                  lambda ob=ob, c2=c2: emit_qt_chunk(ob, c2)))

            queue_ob(1)
            queue_ob(2)
            for skb in range(8):
                tasks.append((("vp", skb, 1), 6,
                              lambda skb=skb: emit_vp_chunk(skb, 1)))
            queue_ob(3)
            for skb in range(8, SKB):
                tasks.append((("vp", skb, 1), 6,
                              lambda skb=skb: emit_vp_chunk(skb, 1)))
            queue_ob(4)
            queue_ob(5)
            vp0_tasks = [(("vp", skb, 0), 6,
                          lambda skb=skb: emit_vp_chunk(skb, 0))
                         for skb in range(SKB)]

            state = {"budget": 0.0}

            def force(pred, lst=None):
                lst = tasks if lst is None else lst
                rest = []
                for t in lst:
                    if pred(t[0]):
                        t[2]()
                        state["budget"] -= t[1]
                    else:
                        rest.append(t)
                lst[:] = rest

            def inject(budget_add):
                state["budget"] += budget_add
                while tasks and tasks[0][1] <= state["budget"]:
                    tag, mms, fn = tasks.pop(0)
                    fn()
                    state["budget"] -= mms

            # ---- prelude: kTz[0,1], qT[0] ----
            for c4 in range(4):
                emit_kt_chunk(0, c4)
            for c2 in range(2):
                emit_qt_chunk(0, c2)

            # ---- attention pipeline (flat skb stream, PV lags QK by 2) ----
            pt_live = {}

            def emit_qk(h, s):
                hb = h // 2
                st = mm_ps.tile([128, SQ], f32, name="mmps", tag="mmps",
                                padded_shape=[128, SQ])
                for j in range(2):
                    nc.tensor.matmul(
                        st[:, j * 512:(j + 1) * 512],
                        kTz[h][:, s * 128:(s + 1) * 128],
                        qT[hb][:, j * 512:(j + 1) * 512],
                        start=True, stop=True)
                pt = pt_pool.tile([128, SQ], bf16, name="pt16")
                nc.scalar.activation(pt[:, :], st[:, :], EXP, scale=0.125)
                pt_live[(h, s)] = pt

            vt_live = {}

            def emit_pv(h, s):
                if s == 0:
                    vt_live[h] = vt_ps.tile([65, SQ], f32, name="valT",
                                            tag="valT",
                                            padded_shape=[128, SQ])
                vt = vt_live[h]
                pt = pt_live.pop((h, s))
                ch = 0 if h < 6 else 1
                force(lambda t: t[0] == "vp" and t[1] == s and t[2] == ch,
                      vp0_tasks if ch == 0 else tasks)
                for j in range(2):
                    nc.tensor.matmul(
                        vt[:, j * 512:(j + 1) * 512],
                        vp16[s][:, h * 65:h * 65 + 65],
                        pt[:, j * 512:(j + 1) * 512],
                        start=(s == 0), stop=(s == SKB - 1))

            def emit_vnorm(h):
                hb, r0 = h // 2, (h % 2) * 64
                vt = vt_live.pop(h)
                vals = nrm_pool.tile([64, SQ], f32, name="vals")
                nc.vector.tensor_copy(vals[:, :], vt[0:64, :])
                den = nrm_pool.tile([1, SQ], f32, name="den")
                nc.vector.tensor_copy(den[:, :], vt[64:65, :])
                rec = nrm_pool.tile([1, SQ], f32, name="rec")
                # denominators are positive and well inside normal fp32 range;
                # the approx-fast custom op needs a partition-aligned source,
                # hence the den bounce off partition 64
                nc.vector.reciprocal_approx_fast(rec[:, :], den[:, :])
                rbc = nrm_pool.tile([64, SQ], f32, name="rbc")
                nc.gpsimd.partition_broadcast(rbc[:, :], rec[:, :])
                nc.vector.tensor_mul(
                    vnorm[hb][r0:r0 + 64, :], vals[:, :], rbc[:, :])

            NS = H * SKB  # 192 (h, s) units
            LAG = 2
            for u in range(NS + LAG):
                if u < NS:
                    h2, s2 = divmod(u, SKB)
                    if s2 == 0:
                        force(lambda t, hb2=h2 // 2:
                              t[0] in ("kt", "qt") and t[1] == hb2)
                    emit_qk(h2, s2)
                    inject(1.05)
                if u >= LAG:
                    h1, s1 = divmod(u - LAG, SKB)
                    emit_pv(h1, s1)
                    if s1 == SKB - 1:
                        emit_vnorm(h1)
                    inject(1.05)

            force(lambda t: True)

            # ---- output projection ----
            for sqb in range(SQB):
                if sqb % 3 < 2:
                    op = mm_ps.tile([128, D], f32, name="mmps", tag="mmps",
                                    padded_shape=[128, SQ])
                else:
                    op = vt_ps.tile([128, D], f32, name="valT", tag="valT",
                                    padded_shape=[128, SQ])
                for kb in range(DB):
                    for nc2 in range(2):
                        n0, n1 = nc2 * 512, min(D, (nc2 + 1) * 512)
                        nc.tensor.matmul(
                            op[:, n0:n1],
                            vnorm[kb][:, sqb * 128:(sqb + 1) * 128],
                            wo16t[:, kb, n0:n1],
                            start=(kb == 0), stop=(kb == DB - 1))
                ot = o_pool.tile([128, D], f32, name="osb")
                nc.vector.tensor_copy(ot[:, :], op[:, :])
                nc.sync.dma_start(
                    out=out[sqb * 128:(sqb + 1) * 128, :], in_=ot[:, :])

    nc.compile()
    return nc


def _get_nc():
    if "nc" not in _cache:
        _cache["nc"] = _build_nc()
    return _cache["nc"]


def _host_fallback(x, y, mask, Wq, bq, Wkv, bkv, Wo, bo):
    Bb, Ss, _ = x.shape
    q = x @ Wq.T + bq
    kv = y @ Wkv.T + bkv
    q = q.reshape(Bb, Ss, H, Dh).transpose(0, 2, 1, 3)
    kv = kv.reshape(Bb, Ss, H, 2 * Dh).transpose(0, 2, 1, 3)
    k, v = kv[..., :Dh], kv[..., Dh:]
    scaled = np.einsum("bhqd,bhkd->bhqk", q, k) / np.sqrt(np.float32(Dh))
    scaled = scaled + mask
    scaled -= scaled.max(axis=-1, keepdims=True)
    e = np.exp(scaled)
    attn = e / e.sum(axis=-1, keepdims=True)
    values = np.einsum("bhqk,bhkd->bhqd", attn, v)
    values = values.transpose(0, 2, 1, 3).reshape(Bb, Ss, H * Dh)
    return (values @ Wo.T + bo).astype(np.float32)


def _blk(mat_t, dtype):
    """[768, N] row-blocked to [128, 6, N] in the given ml dtype."""
    n = mat_t.shape[1]
    return np.ascontiguousarray(
        mat_t.reshape(DB, 128, n).transpose(1, 0, 2)).astype(dtype)


def _run(inputs, trace=False, trace_cores=None):
    """Returns (full_output, BassKernelResults)."""
    import ml_dtypes
    from concourse.bass_utils import run_bass_kernel_spmd

    bf16 = ml_dtypes.bfloat16

    x = np.ascontiguousarray(np.asarray(inputs["x"], dtype=np.float32))
    y = np.ascontiguousarray(np.asarray(inputs["y"], dtype=np.float32))
    Wq = np.asarray(inputs["Wq"], dtype=np.float32)
    Wkv = np.asarray(inputs["Wkv"], dtype=np.float32)
    Wo = np.asarray(inputs["Wo"], dtype=np.float32)

    # Reference reshapes kv to [B,S,H,2*Dh]: per head, rows h*128..h*128+63 of
    # Wkv are the k-projection, rows h*128+64..h*128+127 the v-projection.
    k_rows = np.concatenate([np.arange(h * 128, h * 128 + Dh) for h in range(H)])
    v_rows = np.concatenate([np.arange(h * 128 + Dh, (h + 1) * 128)
                             for h in range(H)])
    wq16 = _blk(Wq.T, bf16)
    wk16 = _blk(Wkv[k_rows].T, bf16)
    wv16 = _blk(Wkv[v_rows].T, bf16)
    wo16 = _blk(Wo.T, bf16)

    in_maps = []
    for c in range(N_CORES):
        b, half = c // 2, c % 2
        xT = x[b, half * SQ:(half + 1) * SQ, :].T
        yT = y[b].T
        in_maps.append({
            "x16": _blk(xT, bf16),
            "y16": _blk(yT, bf16),
            "wq16": wq16, "wk16": wk16, "wv16": wv16, "wo16": wo16,
        })

    nc = _get_nc()
    res = run_bass_kernel_spmd(nc, in_maps, core_ids=list(range(N_CORES)),
                               trace=trace, trace_cores=trace_cores)
    out = np.empty((B, S, D), dtype=np.float32)
    for c in range(N_CORES):
        b, half = c // 2, c % 2
        out[b, half * SQ:(half + 1) * SQ, :] = res.results[c]["out"]
    return out, res


def kernel(**inputs) -> np.ndarray:
    mask = np.asarray(inputs["mask"], dtype=np.float32)
    bq = np.asarray(inputs["bq"], dtype=np.float32)
    bkv = np.asarray(inputs["bkv"], dtype=np.float32)
    bo = np.asarray(inputs["bo"], dtype=np.float32)
    if mask.any() or bq.any() or bkv.any() or bo.any():
        # Device kernel hardcodes zero mask/biases; stay correct regardless.
        return _host_fallback(
            np.asarray(inputs["x"], dtype=np.float32),
            np.asarray(inputs["y"], dtype=np.float32),
            mask, np.asarray(inputs["Wq"], dtype=np.float32), bq,
            np.asarray(inputs["Wkv"], dtype=np.float32), bkv,
            np.asarray(inputs["Wo"], dtype=np.float32), bo)
    out, _ = _run(inputs)
    return out


# revision 26
# speedup vs baseline: 1.1898x; 1.1898x over previous
"""Multi-head cross-attention Trainium2 kernel (8 NeuronCores, SPMD).

Problem: nn_MultiHeadCrossAttention_31791347925263
  x:[4,2048,768], y:[4,2048,768], 12 heads x 64, fp32.
  out = softmax((x Wq^T)(y Wk^T)^T / 8 + mask) (y Wv^T) Wo^T   (+ zero biases)

Sharding: 8 cores = (batch b in 0..3) x (query half in 0..1). Each core
computes the full attention for its 1024 query rows against all 2048 keys
of its batch. No collectives; outputs concatenate.

Design (v3, ACT-limited pipeline, all-bf16):
  Measured HW laws driving this shape:
  - A K=128 matmul streams 512 moving cols in ~226ns (full 2.4GHz);
    K<=64 matmuls run exactly 2x slower. So QK (contraction = head_dim
    = 64) uses ZERO-PADDED stationaries: kTz[h] is [128, 2048] with the
    head's k in rows 0-63 and zeros in 64-127; the moving qT block has
    the sibling head's (finite) q in rows 64-127, killed by the zeros.
    This halves QK's PE time.
  - fp8 anywhere in the PV chain costs ~2-3% output error (softmax
    output rel err ~= per-element rel err of P~/v; it does NOT average
    down), so everything stays bf16 (~0.7% total).
  - The Scalar engine exp (25.2M scores -> 192 x [128,1024] ACTIVATEs
    at ~1.1us) is a ~214us floor. PE work (614k cols ~= 256us at full
    clock) is brought to ~82+82+15us attention-side by the K=128 trick,
    with the 77us of kT/qT/v' projections injected into per-pair PE
    slack so ACT never starves. PSUM: 2x QK score slots [128,1024] (4
    banks) + PV accumulator [65,1024] (2) + 2 projection slots (2).
  - Single-head pipeline with QK emitted 2 key-blocks ahead of PV so
    the in-order PE queue never blocks on exp; projection chunks are
    force-drained before their consumers (deadlock safety).
  - PV's 65th stationary column (ones) accumulates the softmax
    denominator free; normalize = DVE copy + reciprocal + gpsimd
    partition-broadcast + DVE mul into separate bf16 vnorm tiles.
  - Output projection (bf16) at the end over 3 rotating PSUM slots.
"""

import numpy as np

B, S, D = 4, 2048, 768
H, Dh = 12, 64
SQ = S // 2          # queries per core
N_CORES = 8
DB = D // 128        # 6 d_model blocks
SKB = S // 128       # 16 key blocks
SQB = SQ // 128      # 8 query blocks per core
VPW = H * (Dh + 1)   # 780: v' width (64 v cols + 1 ones col per head)

_cache = {}


def _build_nc():
    import concourse.mybir as mybir
    import concourse.tile as tile
    from concourse import bacc

    f32 = mybir.dt.float32
    bf16 = mybir.dt.bfloat16
    EXP = mybir.ActivationFunctionType.Exp

    nc = bacc.Bacc("TRN2", target_bir_lowering=False)
    x16 = nc.dram_tensor("x16", [128, DB, SQ], bf16, kind="ExternalInput")
    y16 = nc.dram_tensor("y16", [128, DB, S], bf16, kind="ExternalInput")
    wq16 = nc.dram_tensor("wq16", [128, DB, D], bf16, kind="ExternalInput")
    wk16 = nc.dram_tensor("wk16", [128, DB, D], bf16, kind="ExternalInput")
    wv16 = nc.dram_tensor("wv16", [128, DB, D], bf16, kind="ExternalInput")
    wo16 = nc.dram_tensor("wo16", [128, DB, D], bf16, kind="ExternalInput")
    out = nc.dram_tensor("out", [SQ, D], f32, kind="ExternalOutput")

    with tile.TileContext(nc) as tc:
        with tc.tile_pool(name="persist", bufs=1) as pp, \
             tc.tile_pool(name="mmps", bufs=2, space="PSUM") as mm_ps, \
             tc.tile_pool(name="vtps", bufs=1, space="PSUM") as vt_ps, \
             tc.tile_pool(name="pjps", bufs=2, space="PSUM") as pj_ps, \
             tc.tile_pool(name="pt16p", bufs=4) as pt_pool, \
             tc.tile_pool(name="nrm", bufs=1) as nrm_pool, \
             tc.tile_pool(name="osb", bufs=3) as o_pool:

            y16t = pp.tile([128, DB, S], bf16, name="y16t")
            wk16t = pp.tile([128, DB, D], bf16, name="wk16t")
            x16t = pp.tile([128, DB, SQ], bf16, name="x16t")
            wq16t = pp.tile([128, DB, D], bf16, name="wq16t")
            wv16t = pp.tile([128, DB, D], bf16, name="wv16t")
            wo16t = pp.tile([128, DB, D], bf16, name="wo16t")

            # zero-padded per-head k: rows 0-63 = head's kT, 64-127 = 0
            kTz = [pp.tile([128, S], bf16, name=f"kTz{i}") for i in range(H)]
            qT = [pp.tile([128, SQ], bf16, name=f"qT{i}") for i in range(DB)]
            vnorm = [pp.tile([128, SQ], bf16, name=f"vn{i}")
                     for i in range(DB)]
            vp16 = [pp.tile([128, VPW], bf16, name=f"vp16_{i}")
                    for i in range(SKB)]
            vp3 = [t.rearrange("p (h c) -> p h c", c=Dh + 1) for t in vp16]

            # ---- input DMA, priority order ----
            nc.sync.dma_start(out=wk16t, in_=wk16[:, :, :])
            for kb in range(DB):
                nc.sync.dma_start(out=y16t[:, kb, :], in_=y16[:, kb, :])
            nc.sync.dma_start(out=wq16t, in_=wq16[:, :, :])
            for kb in range(DB):
                nc.sync.dma_start(out=x16t[:, kb, :], in_=x16[:, kb, :])
            nc.sync.dma_start(out=wv16t, in_=wv16[:, :, :])
            nc.sync.dma_start(out=wo16t, in_=wo16[:, :, :])

            # head h's k occupies the same partition rows as its q in qT:
            # even heads rows 0-63 (zeros 64-127), odd heads rows 64-127
            for h in range(H):
                z0 = 64 if h % 2 == 0 else 0
                nc.gpsimd.memset(kTz[h][z0:z0 + 64, :], 0.0)
            for skb in range(SKB):
                nc.vector.memset(vp3[skb][:, :, Dh], 1.0)

            # ---- projection chunk emitters ----
            def emit_kt_chunk(ob, c4):
                ps = pj_ps.tile([128, 512], f32, name="pjps", tag="pjps")
                for kb in range(DB):
                    nc.tensor.matmul(
                        ps[:, :],
                        wk16t[:, kb, ob * 128:(ob + 1) * 128],
                        y16t[:, kb, c4 * 512:(c4 + 1) * 512],
                        start=(kb == 0), stop=(kb == DB - 1))
                cols = slice(c4 * 512, (c4 + 1) * 512)
                nc.vector.tensor_copy(kTz[2 * ob][0:64, cols], ps[0:64, :])
                nc.vector.tensor_copy(kTz[2 * ob + 1][64:128, cols],
                                      ps[64:128, :])

            def emit_qt_chunk(ob, c2):
                ps = pj_ps.tile([128, 512], f32, name="pjps", tag="pjps")
                for kb in range(DB):
                    nc.tensor.matmul(
                        ps[:, :],
                        wq16t[:, kb, ob * 128:(ob + 1) * 128],
                        x16t[:, kb, c2 * 512:(c2 + 1) * 512],
                        start=(kb == 0), stop=(kb == DB - 1))
                nc.vector.tensor_copy(
                    qT[ob][:, c2 * 512:(c2 + 1) * 512], ps[:, :])

            def emit_vp_chunk(skb, c):
                ps = pj_ps.tile([128, 512], f32, name="pjps", tag="pjps")
                for kb in range(DB):
                    nc.tensor.matmul(
                        ps[:, 0:384],
                        y16t[:, kb, skb * 128:(skb + 1) * 128],
                        wv16t[:, kb, c * 384:(c + 1) * 384],
                        start=(kb == 0), stop=(kb == DB - 1))
                src = ps[:, 0:384].rearrange("p (h c) -> p h c", c=Dh)
                nc.vector.tensor_copy(
                    vp3[skb][:, c * 6:(c + 1) * 6, 0:Dh], src)

            # task queue: (tag, mm_count, emit_fn), in need-by order.
            # vp chunk-0 tasks are NOT queued: head 0's PV forces them JIT.
            tasks = []

            def queue_ob(ob):
                for c4 in range(4):
                    tasks.append((("kt", ob, c4), 6,
                                  lambda ob=ob, c4=c4: emit_kt_chunk(ob, c4)))
                for c2 in range(2):
                    tasks.append((("qt", ob), 6,
                                  lambda ob=ob, c2=c2: emit_qt_chunk(ob, c2)))

            for skb in range(SKB):
                tasks.append((("vp", skb, 0), 6,
                              lambda skb=skb: emit_vp_chunk(skb, 0)))
            queue_ob(1)
            queue_ob(2)
            for skb in range(8):
                tasks.append((("vp", skb, 1), 6,
                              lambda skb=skb: emit_vp_chunk(skb, 1)))
            queue_ob(3)
            for skb in range(8, SKB):
                tasks.append((("vp", skb, 1), 6,
                              lambda skb=skb: emit_vp_chunk(skb, 1)))
            queue_ob(4)
            queue_ob(5)

            state = {"budget": 0.0}

            def force(pred, lst=None):
                lst = tasks if lst is None else lst
                rest = []
                for t in lst:
                    if pred(t[0]):
                        t[2]()
                        state["budget"] -= t[1]
                    else:
                        rest.append(t)
                lst[:] = rest

            def inject(budget_add):
                state["budget"] += budget_add
                while tasks and tasks[0][1] <= state["budget"]:
                    tag, mms, fn = tasks.pop(0)
                    fn()
                    state["budget"] -= mms

            # ---- prelude: kTz[0,1], qT[0] ----
            for c4 in range(4):
                emit_kt_chunk(0, c4)
            for c2 in range(2):
                emit_qt_chunk(0, c2)

            # ---- attention pipeline (flat skb stream, PV lags QK by 2) ----
            pt_live = {}

            def emit_qk(h, s):
                hb = h // 2
                st = mm_ps.tile([128, SQ], f32, name="mmps", tag="mmps",
                                padded_shape=[128, SQ])
                for j in range(2):
                    nc.tensor.matmul(
                        st[:, j * 512:(j + 1) * 512],
                        kTz[h][:, s * 128:(s + 1) * 128],
                        qT[hb][:, j * 512:(j + 1) * 512],
                        start=True, stop=True)
                pt = pt_pool.tile([128, SQ], bf16, name="pt16")
                nc.scalar.activation(pt[:, :], st[:, :], EXP, scale=0.125)
                pt_live[(h, s)] = pt

            vt_live = {}

            def emit_pv(h, s):
                if s == 0:
                    vt_live[h] = vt_ps.tile([65, SQ], f32, name="valT",
                                            tag="valT",
                                            padded_shape=[128, SQ])
                vt = vt_live[h]
                pt = pt_live.pop((h, s))
                ch = 0 if h < 6 else 1
                force(lambda t: t[0] == "vp" and t[1] == s and t[2] == ch)
                for j in range(2):
                    nc.tensor.matmul(
                        vt[:, j * 512:(j + 1) * 512],
                        vp16[s][:, h * 65:h * 65 + 65],
                        pt[:, j * 512:(j + 1) * 512],
                        start=(s == 0), stop=(s == SKB - 1))

            def emit_vnorm(h):
                hb, r0 = h // 2, (h % 2) * 64
                vt = vt_live.pop(h)
                vals = nrm_pool.tile([64, SQ], f32, name="vals")
                nc.vector.tensor_copy(vals[:, :], vt[0:64, :])
                den = nrm_pool.tile([1, SQ], f32, name="den")
                nc.vector.tensor_copy(den[:, :], vt[64:65, :])
                rec = nrm_pool.tile([1, SQ], f32, name="rec")
                # denominators are positive and well inside normal fp32 range;
                # the approx-fast custom op needs a partition-aligned source,
                # hence the den bounce off partition 64
                nc.vector.reciprocal_approx_fast(rec[:, :], den[:, :])
                rbc = nrm_pool.tile([64, SQ], f32, name="rbc")
                nc.gpsimd.partition_broadcast(rbc[:, :], rec[:, :])
                nc.vector.tensor_mul(
                    vnorm[hb][r0:r0 + 64, :], vals[:, :], rbc[:, :])

            NS = H * SKB  # 192 (h, s) units
            LAG = 3
            for u in range(NS + LAG):
                if u < NS:
                    h2, s2 = divmod(u, SKB)
                    if s2 == 0:
                        force(lambda t, hb2=h2 // 2:
                              t[0] == "qt" and t[1] == hb2)
                    if s2 % 4 == 0:
                        force(lambda t, hb2=h2 // 2, c4=s2 // 4:
                              t[0] == "kt" and t[1] == hb2 and t[2] == c4)
                    emit_qk(h2, s2)
                    inject(0.95)
                if u >= LAG:
                    h1, s1 = divmod(u - LAG, SKB)
                    emit_pv(h1, s1)
                    if s1 == SKB - 1:
                        emit_vnorm(h1)
                    inject(0.95)

            force(lambda t: True)

            # ---- output projection ----
            for sqb in range(SQB):
                if sqb % 3 < 2:
                    op = mm_ps.tile([128, D], f32, name="mmps", tag="mmps",
                                    padded_shape=[128, SQ])
                else:
                    op = vt_ps.tile([128, D], f32, name="valT", tag="valT",
                                    padded_shape=[128, SQ])
                for kb in range(DB):
                    for nc2 in range(2):
                        n0, n1 = nc2 * 512, min(D, (nc2 + 1) * 512)
                        nc.tensor.matmul(
                            op[:, n0:n1],
                            vnorm[kb][:, sqb * 128:(sqb + 1) * 128],
                            wo16t[:, kb, n0:n1],
                            start=(kb == 0), stop=(kb == DB - 1))
                ot = o_pool.tile([128, D], f32, name="osb")
                nc.vector.tensor_copy(ot[:, :], op[:, :])
                nc.sync.dma_start(
                    out=out[sqb * 128:(sqb + 1) * 128, :], in_=ot[:, :])

    nc.compile()
    return nc


def _get_nc():
    if "nc" not in _cache:
        _cache["nc"] = _build_nc()
    return _cache["nc"]


def _host_fallback(x, y, mask, Wq, bq, Wkv, bkv, Wo, bo):
    Bb, Ss, _ = x.shape
    q = x @ Wq.T + bq
    kv = y @ Wkv.T + bkv
    q = q.reshape(Bb, Ss, H, Dh).transpose(0, 2, 1, 3)
    kv = kv.reshape(Bb, Ss, H, 2 * Dh).transpose(0, 2, 1, 3)
    k, v = kv[..., :Dh], kv[..., Dh:]
    scaled = np.einsum("bhqd,bhkd->bhqk", q, k) / np.sqrt(np.float32(Dh))
    scaled = scaled + mask
    scaled -= scaled.max(axis=-1, keepdims=True)
    e = np.exp(scaled)
    attn = e / e.sum(axis=-1, keepdims=True)
    values = np.einsum("bhqk,bhkd->bhqd", attn, v)
    values = values.transpose(0, 2, 1, 3).reshape(Bb, Ss, H * Dh)
    return (values @ Wo.T + bo).astype(np.float32)


def _blk(mat_t, dtype):
    """[768, N] row-blocked to [128, 6, N] in the given ml dtype."""
    n = mat_t.shape[1]
    return np.ascontiguousarray(
        mat_t.reshape(DB, 128, n).transpose(1, 0, 2)).astype(dtype)


def _run(inputs, trace=False, trace_cores=None):
    """Returns (full_output, BassKernelResults)."""
    import ml_dtypes
    from concourse.bass_utils import run_bass_kernel_spmd

    bf16 = ml_dtypes.bfloat16

    x = np.ascontiguousarray(np.asarray(inputs["x"], dtype=np.float32))
    y = np.ascontiguousarray(np.asarray(inputs["y"], dtype=np.float32))
    Wq = np.asarray(inputs["Wq"], dtype=np.float32)
    Wkv = np.asarray(inputs["Wkv"], dtype=np.float32)
    Wo = np.asarray(inputs["Wo"], dtype=np.float32)

    # Reference reshapes kv to [B,S,H,2*Dh]: per head, rows h*128..h*128+63 of
    # Wkv are the k-projection, rows h*128+64..h*128+127 the v-projection.
    k_rows = np.concatenate([np.arange(h * 128, h * 128 + Dh) for h in range(H)])
    v_rows = np.concatenate([np.arange(h * 128 + Dh, (h + 1) * 128)
                             for h in range(H)])
    wq16 = _blk(Wq.T, bf16)
    wk16 = _blk(Wkv[k_rows].T, bf16)
    wv16 = _blk(Wkv[v_rows].T, bf16)
    wo16 = _blk(Wo.T, bf16)

    in_maps = []
    for c in range(N_CORES):
        b, half = c // 2, c % 2
        xT = x[b, half * SQ:(half + 1) * SQ, :].T
        yT = y[b].T
        in_maps.append({
            "x16": _blk(xT, bf16),
            "y16": _blk(yT, bf16),
            "wq16": wq16, "wk16": wk16, "wv16": wv16, "wo16": wo16,
        })

    nc = _get_nc()
    res = run_bass_kernel_spmd(nc, in_maps, core_ids=list(range(N_CORES)),
                               trace=trace, trace_cores=trace_cores)
    out = np.empty((B, S, D), dtype=np.float32)
    for c in range(N_CORES):
        b, half = c // 2, c % 2
        out[b, half * SQ:(half + 1) * SQ, :] = res.results[c]["out"]
    return out, res


def kernel(**inputs) -> np.ndarray:
    mask = np.asarray(inputs["mask"], dtype=np.float32)
    bq = np.asarray(inputs["bq"], dtype=np.float32)
    bkv = np.asarray(inputs["bkv"], dtype=np.float32)
    bo = np.asarray(inputs["bo"], dtype=np.float32)
    if mask.any() or bq.any() or bkv.any() or bo.any():
        # Device kernel hardcodes zero mask/biases; stay correct regardless.
        return _host_fallback(
            np.asarray(inputs["x"], dtype=np.float32),
            np.asarray(inputs["y"], dtype=np.float32),
            mask, np.asarray(inputs["Wq"], dtype=np.float32), bq,
            np.asarray(inputs["Wkv"], dtype=np.float32), bkv,
            np.asarray(inputs["Wo"], dtype=np.float32), bo)
    out, _ = _run(inputs)
    return out
